# revision 18
# baseline (speedup 1.0000x reference)
# Mixture-of-two-experts (modality-routed) token GEMM on 8 Trainium2 NeuronCores.
#
# reference computes BOTH expert GEMMs and selects per token; only one GEMM per
# token is needed. Strategy (expert-dispatch, per the sharding hint):
#   host: partition tokens by type_id, pad each expert's token list to
#         4 * n_tok rows, transpose to [D, n_tok] fp16 per core chunk.
#   device (SPMD, uniform program): plain GEMM  y[tok, e] = x[tok, :] @ Wt + b
#         with fp16 operands, fp32 PSUM accumulation. Cores 0-3 carry expert-0
#         tokens + W0, cores 4-7 expert-1 tokens + W1 (weights arrive as data,
#         so the per-core program is identical).
#   host: inverse-scatter per-expert outputs back to [B, S, D] fp32.

import os
import sys

import numpy as np

for _p in ("/opt/trn_rl_repo", "/root/.axon_site/_ro/trn_rl_repo"):
    if os.path.isdir(_p) and _p not in sys.path:
        sys.path.insert(0, _p)

import concourse.bacc as bacc
import concourse.mybir as mybir
import concourse.tile as tile
from concourse.bass_utils import run_bass_kernel_spmd

D = 2048
KT = D // 128  # 16 contraction tiles
N_CORES = 8
CORES_PER_EXPERT = 4

_PROGRAM_CACHE: dict[int, object] = {}
LAST_RESULTS = None  # BassKernelResults of the most recent launch (for profiling)


def _build_program(n_tok: int):
    """One NeuronCore program: y[n_tok, D] = xt.T @ wt + bias (fp16 in, fp32 out)."""
    m_tiles = n_tok // 128
    f16 = mybir.dt.float16
    f32 = mybir.dt.float32

    nc = bacc.Bacc("TRN2", target_bir_lowering=False, debug=False, num_devices=N_CORES)
    xt = nc.dram_tensor("xt", [KT, 128, n_tok], f16, kind="ExternalInput").ap()
    wt = nc.dram_tensor("wt", [KT, 128, D], f16, kind="ExternalInput").ap()
    bias = nc.dram_tensor("bias", [128, D], f32, kind="ExternalInput").ap()
    y = nc.dram_tensor("y", [n_tok, D], f32, kind="ExternalOutput").ap()
    y_t = y.rearrange("(m p) e -> m p e", p=128)

    # The PE can only keep 2 full-width PSUM accumulation chains in flight, so
    # during the ~45us operand load it starves between k-tile arrivals. Fix:
    # the first N_SPLIT m-tiles accumulate k=0..7 into SBUF partials as soon as
    # the first half of the k-tiles lands (phase A), and finish k=8..15 later
    # (phase B). Everything else runs the plain full-k walk.
    n_split = 5 if m_tiles >= 8 else 0

    with tile.TileContext(nc) as tc:
        with (
            tc.tile_pool(name="wp", bufs=1) as wp,
            tc.tile_pool(name="xp", bufs=1) as xp,
            tc.tile_pool(name="bp", bufs=1) as bp,
            tc.tile_pool(name="ap", bufs=1) as apool,
            tc.tile_pool(name="op", bufs=3) as op_,
            tc.tile_pool(name="pp", bufs=2, space="PSUM") as pp,
        ):
            # Whole operand set fits in SBUF (~204 KiB/partition with the
            # partial accumulators); per-k tiles so matmuls start as soon as
            # the first slices land. Bias is loaded last - it is first needed
            # ~25us in, after several k-tiles.
            # x tiles are split at the phase-A/B boundary: the "head" columns
            # (m-tiles 0..n_split-1) are what the early split-k chains need,
            # so loading all heads+weights first (672 KB per k instead of
            # 1.07 MB) matches the PE's ~1.7us-per-k-pair demand during the
            # load window. Tails follow; they are only needed by the full
            # m-tile walks that start much later.
            head = n_split * 128
            xh, xtl, wk = [], [], []
            bias_s = bp.tile([128, D], f32, name="bias_s")
            for k in range(KT):
                ws = wp.tile([128, D], f16, name=f"w{k}", tag=f"w{k}")
                nc.sync.dma_start(ws[:], wt[k])
                wk.append(ws)
                if n_split:
                    h = xp.tile([128, head], f16, name=f"xh{k}", tag=f"xh{k}")
                    nc.sync.dma_start(h[:], xt[k][:, 0:head])
                    xh.append(h)
            nc.sync.dma_start(bias_s[:], bias[:])
            for k in range(KT):
                t = xp.tile([128, n_tok - head], f16, name=f"xt{k}", tag=f"xt{k}")
                nc.sync.dma_start(t[:], xt[k][:, head:n_tok])
                xtl.append(t)

            def lhs_slice(k, m):
                if m < n_split:
                    return xh[k][:, m * 128 : (m + 1) * 128]
                return xtl[k][:, (m - n_split) * 128 : (m - n_split + 1) * 128]

            # PE warm-up: matmuls on a zeroed tile, no DMA dependency. Runs
            # during the DMA ramp (PE would idle anyway) and flips the HAM
            # clock gate to 8/8 before the first real matmul.
            wz = bp.tile([128, 512], f16, name="wz")
            nc.gpsimd.memset(wz[:], 0.0)
            psw = pp.tile([128, 512], f32, name="psw", tag="ps")
            for _ in range(16):
                nc.tensor.matmul(psw[:], wz[:, 0:128], wz[:], start=True, stop=True)

            def mm_chain(ps, m, ks):
                first = last = None
                for j, k in enumerate(ks):
                    lhsT = lhs_slice(k, m)  # [K, M] stationary
                    for c in range(4):
                        mm = nc.tensor.matmul(
                            ps[:, c * 512 : (c + 1) * 512],
                            lhsT,
                            wk[k][:, c * 512 : (c + 1) * 512],
                            start=(j == 0),
                            stop=(j == len(ks) - 1),
                        )
                        first = first or mm
                        last = mm
                return first, last

            def drain(ps, addend, m, chunked_dma=False):
                ot = op_.tile([128, D], f32, name=f"ot{m}", tag="ot")
                for c in range(4):
                    sl = slice(c * 512, (c + 1) * 512)
                    nc.vector.tensor_add(ot[:, sl], ps[:, sl], addend[:, sl])
                    if chunked_dma:
                        nc.sync.dma_start(y_t[m][:, sl], ot[:, sl])
                if not chunked_dma:
                    nc.sync.dma_start(y_t[m], ot[:])

            prev_last = None

            def pin(first, reason):
                # keep the PE stream in emission order chain-by-chain: the
                # scheduler otherwise hoists later chains (gated on late k
                # arrivals) ahead of ready work and stalls the PE
                if prev_last is not None:
                    tile.add_dep_helper(
                        first.ins, prev_last.ins, sync=False, reason=reason
                    )

            acc = {}
            for m in range(n_split):  # phase A: k=0..7 -> SBUF partial
                ps = pp.tile([128, D], f32, name=f"psa{m}", tag="ps")
                fa, la = mm_chain(ps, m, range(KT // 2))
                pin(fa, f"chain order A{m}")
                prev_last = la
                # no bias here: bias is the LAST DMA and must not gate the
                # phase-A psum drains (it would stall the PE for ~25us)
                a = apool.tile([128, D], f32, name=f"acc{m}", tag=f"acc{m}")
                for c in range(4):
                    sl = slice(c * 512, (c + 1) * 512)
                    nc.vector.tensor_copy(a[:, sl], ps[:, sl])
                acc[m] = a

            for m in range(n_split):  # phase B: k=8..15 + partial + bias
                ps = pp.tile([128, D], f32, name=f"psb{m}", tag="ps")
                fb, lb = mm_chain(ps, m, range(KT // 2, KT))
                pin(fb, f"chain order B{m}")
                prev_last = lb
                ot = op_.tile([128, D], f32, name=f"otb{m}", tag="ot")
                for c in range(4):
                    sl = slice(c * 512, (c + 1) * 512)
                    nc.vector.tensor_add(ot[:, sl], ps[:, sl], acc[m][:, sl])
                    nc.vector.tensor_add(ot[:, sl], ot[:, sl], bias_s[:, sl])
                nc.sync.dma_start(y_t[m], ot[:])

            full = list(range(n_split, m_tiles))
            for m in full:
                ps = pp.tile([128, D], f32, name=f"ps{m}", tag="ps")
                ff, lf = mm_chain(ps, m, range(KT))
                pin(ff, f"chain order F{m}")
                prev_last = lf
                drain(ps, bias_s, m, chunked_dma=(m == full[-1]))

    nc.compile()
    return nc


def _get_program(n_tok: int):
    if n_tok not in _PROGRAM_CACHE:
        _PROGRAM_CACHE[n_tok] = _build_program(n_tok)
    return _PROGRAM_CACHE[n_tok]


def _round_up(v: int, m: int) -> int:
    return -(-v // m) * m


def kernel(hidden_states, type_ids, W0, b0, W1, b1, _trace=False, _tmpdir=None):
    global LAST_RESULTS

    B, S, D_ = hidden_states.shape
    assert D_ == D
    x = np.ascontiguousarray(np.asarray(hidden_states, dtype=np.float32)).reshape(
        B * S, D
    )
    t = np.asarray(type_ids).reshape(B * S)

    idx = [np.nonzero(t == e)[0] for e in (0, 1)]
    counts = [len(i) for i in idx]
    # tokens per core: 4 cores per expert, padded to 128-token tiles
    n_tok = max(128, _round_up(-(-max(counts) // CORES_PER_EXPERT), 128))
    cap = n_tok * CORES_PER_EXPERT

    nc = _get_program(n_tok)

    wts, biases = [], []
    for W, b in ((W0, b0), (W1, b1)):
        wts.append(
            np.ascontiguousarray(
                np.asarray(W, dtype=np.float32).T.astype(np.float16)
            ).reshape(KT, 128, D)
        )
        biases.append(
            np.ascontiguousarray(
                np.broadcast_to(np.asarray(b, dtype=np.float32), (128, D))
            )
        )

    in_maps = []
    for e in (0, 1):
        g = x[idx[e]].astype(np.float16)  # [count_e, D]
        if g.shape[0] < cap:
            g = np.concatenate(
                [g, np.zeros((cap - g.shape[0], D), np.float16)], axis=0
            )
        for c in range(CORES_PER_EXPERT):
            chunk = g[c * n_tok : (c + 1) * n_tok]  # [n_tok, D]
            xt_c = np.ascontiguousarray(chunk.T).reshape(KT, 128, n_tok)
            in_maps.append({"xt": xt_c, "wt": wts[e], "bias": biases[e]})

    res = run_bass_kernel_spmd(
        nc, in_maps, list(range(N_CORES)), trace=_trace, tmpdir=_tmpdir
    )
    LAST_RESULTS = res

    out = np.empty((B * S, D), dtype=np.float32)
    for e in (0, 1):
        ys = np.concatenate(
            [res.results[e * CORES_PER_EXPERT + c]["y"] for c in range(CORES_PER_EXPERT)],
            axis=0,
        )
        out[idx[e]] = ys[: counts[e]]
    return out.reshape(B, S, D)
